# revision 8
# baseline (speedup 1.0000x reference)
"""Trainium2 Bass kernel for nn_LogicLayer (ProductTNorm 'and' LogicLayer forward).

Math: y[b,o] = prod_i (1 - (1-atoms[b,i]) * sigmoid(weights[o,i]))
           = exp( sum_i ln( omv[o,i] + v[o,i]*atoms[b,i] ) )
  with v = sigmoid(w), omv = sigmoid(-w) = 1 - v.

Device strategy (8 cores, sharded by OUTPUT FEATURE, 32 o's per core,
atoms replicated):
  * atoms.T lives in SBUF as two (128, 4096) fp32 tiles (i on partitions).
  * For each (o, i-tile): ONE ScalarE activation computes
      logw = Ln(a * v_col + omv_col)   (scale/bias are per-partition APs)
    over the full 4096-wide batch in fp16.
  * TensorE sums logw over the 128 partitions via a one-hot stationary
    (column o = ones) accumulating into PSUM row o, fp32.
  * One final ScalarE Exp over the (32, 4096) PSUM -> y tile -> DMA out.
"""

import os
from contextlib import ExitStack

import numpy as np

B, OUT, IN = 4096, 256, 256
NCORES = 8
O_LOC = OUT // NCORES  # 32 output features per core
PCHUNK = 512  # matmul moving free-dim / PSUM bank size in fp32
LOGW_DT_NAME = os.environ.get("KERNEL_LOGW_DT", "float16")

_COMPILED = {}


def _build_nc():
    import concourse.bacc as bacc
    import concourse.mybir as mybir
    import concourse.tile as tile

    AF = mybir.ActivationFunctionType
    F32 = mybir.dt.float32
    LOGW_DT = getattr(mybir.dt, LOGW_DT_NAME)

    nc = bacc.Bacc(
        "TRN2", target_bir_lowering=False, debug=False, num_devices=NCORES
    )

    aT = nc.dram_tensor("aT", [IN, B], F32, kind="ExternalInput").ap()
    wT = nc.dram_tensor("wT", [IN, O_LOC], F32, kind="ExternalInput").ap()
    sel = nc.dram_tensor("sel", [128, O_LOC * O_LOC], LOGW_DT, kind="ExternalInput").ap()
    y = nc.dram_tensor("y", [O_LOC, B], F32, kind="ExternalOutput").ap()

    NIT = IN // 128  # 2 i-tiles
    NK = B // PCHUNK  # 8 psum bank chunks

    with tile.TileContext(nc) as tc, ExitStack() as es:
        const = es.enter_context(tc.tile_pool(name="const", bufs=1))
        lw_pool = es.enter_context(tc.tile_pool(name="lw", bufs=4))
        ps_pool = es.enter_context(tc.tile_pool(name="ps", bufs=1, space="PSUM"))

        # Small inputs first: weights + selector, so sigmoids can run while
        # the big atoms DMA streams in.
        w_sb = const.tile([128, NIT * O_LOC], F32, name="w_sb", tag="w_sb")
        for it in range(NIT):
            nc.sync.dma_start(
                w_sb[:, it * O_LOC : (it + 1) * O_LOC],
                wT[it * 128 : (it + 1) * 128, :],
            )
        sel_sb = const.tile([128, O_LOC * O_LOC], LOGW_DT, name="sel_sb", tag="sel_sb")
        nc.sync.dma_start(sel_sb[:], sel[:])

        v_sb = const.tile([128, NIT * O_LOC], F32, name="v_sb", tag="v_sb")
        omv_sb = const.tile([128, NIT * O_LOC], F32, name="omv_sb", tag="omv_sb")
        nc.scalar.activation(v_sb[:], w_sb[:], AF.Sigmoid)
        nc.scalar.activation(omv_sb[:], w_sb[:], AF.Sigmoid, scale=-1.0)

        # Atoms: column-chunk DMAs spread across the HW-DGE (sync) and
        # SW-DGE (gpsimd) rings so queues run in parallel and the first Ln
        # can start sooner.
        ACH = B // 8
        a_sb = []
        for it in range(NIT):
            t = const.tile([128, B], F32, name=f"a_sb{it}", tag=f"a_sb{it}")
            for q in range(8):
                eng = nc.sync if q % 2 == 0 else nc.gpsimd
                eng.dma_start(
                    t[:, q * ACH : (q + 1) * ACH],
                    aT[it * 128 : (it + 1) * 128, q * ACH : (q + 1) * ACH],
                )
            a_sb.append(t)

        psum = ps_pool.tile([O_LOC, B], F32, name="psum_S", tag="psum_S")

        for it in range(NIT):
            for o in range(O_LOC):
                c = it * O_LOC + o
                lw = lw_pool.tile([128, B], LOGW_DT, name="lw", tag="lw")
                if it == 0 and o == 0:
                    # Split the first Ln so it can start as soon as the
                    # first half of the atoms tile has landed.
                    for h in range(2):
                        hs = slice(h * (B // 2), (h + 1) * (B // 2))
                        nc.scalar.activation(
                            lw[:, hs],
                            a_sb[it][:, hs],
                            AF.Ln,
                            bias=omv_sb[:, c : c + 1],
                            scale=v_sb[:, c : c + 1],
                        )
                else:
                    nc.scalar.activation(
                        lw[:],
                        a_sb[it][:],
                        AF.Ln,
                        bias=omv_sb[:, c : c + 1],
                        scale=v_sb[:, c : c + 1],
                    )
                for k in range(NK):
                    nc.tensor.matmul(
                        psum[:, k * PCHUNK : (k + 1) * PCHUNK],
                        lhsT=sel_sb[:, o * O_LOC : (o + 1) * O_LOC],
                        rhs=lw[:, k * PCHUNK : (k + 1) * PCHUNK],
                        start=(it == 0 and o == 0),
                        stop=(it == NIT - 1 and o == O_LOC - 1),
                    )

        # Tail: pipeline Exp chunks with output DMA chunks.
        YCH = B // 2
        y_sb = const.tile([O_LOC, B], F32, name="y_sb", tag="y_sb")
        for q in range(2):
            sl = slice(q * YCH, (q + 1) * YCH)
            nc.scalar.activation(y_sb[:, sl], psum[:, sl], AF.Exp)
            nc.sync.dma_start(y[:, sl], y_sb[:, sl])

    nc.compile()
    return nc


def get_nc():
    if "nc" not in _COMPILED:
        _COMPILED["nc"] = _build_nc()
    return _COMPILED["nc"]


def make_in_maps(atoms: np.ndarray, weights: np.ndarray):
    sel_dt = np.float16 if LOGW_DT_NAME == "float16" else np.float32
    aT = np.ascontiguousarray(atoms.T.astype(np.float32, copy=False))
    sel = np.zeros((128, O_LOC * O_LOC), sel_dt)
    for o in range(O_LOC):
        sel[:, o * O_LOC + o] = 1.0
    in_maps = []
    for c in range(NCORES):
        wT = np.ascontiguousarray(
            weights[c * O_LOC : (c + 1) * O_LOC].T.astype(np.float32, copy=False)
        )
        in_maps.append({"aT": aT, "wT": wT, "sel": sel})
    return in_maps


def run(atoms: np.ndarray, weights: np.ndarray, **spmd_kwargs):
    from concourse.bass_utils import run_bass_kernel_spmd

    nc = get_nc()
    in_maps = make_in_maps(atoms, weights)
    res = run_bass_kernel_spmd(nc, in_maps, core_ids=list(range(NCORES)), **spmd_kwargs)
    yT = np.concatenate([res.results[c]["y"] for c in range(NCORES)], axis=0)
    out = np.ascontiguousarray(yT.T).astype(np.float32, copy=False)
    return out, res


def kernel(atoms: np.ndarray, weights: np.ndarray) -> np.ndarray:
    out, _ = run(atoms, weights)
    return out


# revision 9
# speedup vs baseline: 1.0059x; 1.0059x over previous
"""Trainium2 Bass kernel for nn_LogicLayer (ProductTNorm 'and' LogicLayer forward).

Math: y[b,o] = prod_i (1 - (1-atoms[b,i]) * sigmoid(weights[o,i]))
           = exp( sum_i ln( omv[o,i] + v[o,i]*atoms[b,i] ) )
  with v = sigmoid(w), omv = sigmoid(-w) = 1 - v.

Device strategy (8 cores, sharded by OUTPUT FEATURE, 32 o's per core,
atoms replicated):
  * atoms.T lives in SBUF as two (128, 4096) fp32 tiles (i on partitions).
  * For each (o, i-tile): ONE ScalarE activation computes
      logw = Ln(a * v_col + omv_col)   (scale/bias are per-partition APs)
    over the full 4096-wide batch in fp16.
  * TensorE sums logw over the 128 partitions via a one-hot stationary
    (column o = ones) accumulating into PSUM row o, fp32.
  * One final ScalarE Exp over the (32, 4096) PSUM -> y tile -> DMA out.
"""

import os
from contextlib import ExitStack

import numpy as np

B, OUT, IN = 4096, 256, 256
NCORES = 8
O_LOC = OUT // NCORES  # 32 output features per core
PCHUNK = 512  # matmul moving free-dim / PSUM bank size in fp32
LOGW_DT_NAME = os.environ.get("KERNEL_LOGW_DT", "float16")

_COMPILED = {}


def _build_nc():
    import concourse.bacc as bacc
    import concourse.mybir as mybir
    import concourse.tile as tile

    AF = mybir.ActivationFunctionType
    F32 = mybir.dt.float32
    LOGW_DT = getattr(mybir.dt, LOGW_DT_NAME)

    nc = bacc.Bacc(
        "TRN2", target_bir_lowering=False, debug=False, num_devices=NCORES
    )

    aT = nc.dram_tensor("aT", [IN, B], F32, kind="ExternalInput").ap()
    wT = nc.dram_tensor("wT", [IN, O_LOC], F32, kind="ExternalInput").ap()
    sel = nc.dram_tensor("sel", [128, O_LOC * O_LOC], LOGW_DT, kind="ExternalInput").ap()
    y = nc.dram_tensor("y", [O_LOC, B], F32, kind="ExternalOutput").ap()

    NIT = IN // 128  # 2 i-tiles
    NK = B // PCHUNK  # 8 psum bank chunks

    with tile.TileContext(nc) as tc, ExitStack() as es:
        const = es.enter_context(tc.tile_pool(name="const", bufs=1))
        lw_pool = es.enter_context(tc.tile_pool(name="lw", bufs=4))
        ps_pool = es.enter_context(tc.tile_pool(name="ps", bufs=1, space="PSUM"))

        # Small inputs first: weights + selector, so sigmoids can run while
        # the big atoms DMA streams in.
        w_sb = const.tile([128, NIT * O_LOC], F32, name="w_sb", tag="w_sb")
        for it in range(NIT):
            nc.sync.dma_start(
                w_sb[:, it * O_LOC : (it + 1) * O_LOC],
                wT[it * 128 : (it + 1) * 128, :],
            )
        sel_sb = const.tile([128, O_LOC * O_LOC], LOGW_DT, name="sel_sb", tag="sel_sb")
        nc.sync.dma_start(sel_sb[:], sel[:])

        v_sb = const.tile([128, NIT * O_LOC], F32, name="v_sb", tag="v_sb")
        omv_sb = const.tile([128, NIT * O_LOC], F32, name="omv_sb", tag="omv_sb")
        nc.scalar.activation(v_sb[:], w_sb[:], AF.Sigmoid)
        nc.scalar.activation(omv_sb[:], w_sb[:], AF.Sigmoid, scale=-1.0)

        # Atoms: 4 column-chunk DMAs per i-tile so multiple DMA queues run
        # in parallel and the first Ln can start sooner.
        ACH = B // 4
        a_sb = []
        for it in range(NIT):
            t = const.tile([128, B], F32, name=f"a_sb{it}", tag=f"a_sb{it}")
            for q in range(4):
                nc.sync.dma_start(
                    t[:, q * ACH : (q + 1) * ACH],
                    aT[it * 128 : (it + 1) * 128, q * ACH : (q + 1) * ACH],
                )
            a_sb.append(t)

        psum = ps_pool.tile([O_LOC, B], F32, name="psum_S", tag="psum_S")

        for it in range(NIT):
            for o in range(O_LOC):
                c = it * O_LOC + o
                lw = lw_pool.tile([128, B], LOGW_DT, name="lw", tag="lw")
                if it == 0 and o == 0:
                    # Split the first Ln so it can start as soon as the
                    # first half of the atoms tile has landed.
                    for h in range(2):
                        hs = slice(h * (B // 2), (h + 1) * (B // 2))
                        nc.scalar.activation(
                            lw[:, hs],
                            a_sb[it][:, hs],
                            AF.Ln,
                            bias=omv_sb[:, c : c + 1],
                            scale=v_sb[:, c : c + 1],
                        )
                else:
                    nc.scalar.activation(
                        lw[:],
                        a_sb[it][:],
                        AF.Ln,
                        bias=omv_sb[:, c : c + 1],
                        scale=v_sb[:, c : c + 1],
                    )
                for k in range(NK):
                    nc.tensor.matmul(
                        psum[:, k * PCHUNK : (k + 1) * PCHUNK],
                        lhsT=sel_sb[:, o * O_LOC : (o + 1) * O_LOC],
                        rhs=lw[:, k * PCHUNK : (k + 1) * PCHUNK],
                        start=(it == 0 and o == 0),
                        stop=(it == NIT - 1 and o == O_LOC - 1),
                    )

        # Tail: pipeline Exp chunks with output DMA chunks.
        YCH = B // 2
        y_sb = const.tile([O_LOC, B], F32, name="y_sb", tag="y_sb")
        for q in range(2):
            sl = slice(q * YCH, (q + 1) * YCH)
            nc.scalar.activation(y_sb[:, sl], psum[:, sl], AF.Exp)
            nc.sync.dma_start(y[:, sl], y_sb[:, sl])

    nc.compile()
    return nc


def get_nc():
    if "nc" not in _COMPILED:
        _COMPILED["nc"] = _build_nc()
    return _COMPILED["nc"]


def make_in_maps(atoms: np.ndarray, weights: np.ndarray):
    sel_dt = np.float16 if LOGW_DT_NAME == "float16" else np.float32
    aT = np.ascontiguousarray(atoms.T.astype(np.float32, copy=False))
    sel = np.zeros((128, O_LOC * O_LOC), sel_dt)
    for o in range(O_LOC):
        sel[:, o * O_LOC + o] = 1.0
    in_maps = []
    for c in range(NCORES):
        wT = np.ascontiguousarray(
            weights[c * O_LOC : (c + 1) * O_LOC].T.astype(np.float32, copy=False)
        )
        in_maps.append({"aT": aT, "wT": wT, "sel": sel})
    return in_maps


def run(atoms: np.ndarray, weights: np.ndarray, **spmd_kwargs):
    from concourse.bass_utils import run_bass_kernel_spmd

    nc = get_nc()
    in_maps = make_in_maps(atoms, weights)
    res = run_bass_kernel_spmd(nc, in_maps, core_ids=list(range(NCORES)), **spmd_kwargs)
    yT = np.concatenate([res.results[c]["y"] for c in range(NCORES)], axis=0)
    out = np.ascontiguousarray(yT.T).astype(np.float32, copy=False)
    return out, res


def kernel(atoms: np.ndarray, weights: np.ndarray) -> np.ndarray:
    out, _ = run(atoms, weights)
    return out


# revision 10
# speedup vs baseline: 1.1821x; 1.1752x over previous
"""Trainium2 Bass kernel for nn_LogicLayer (ProductTNorm 'and' LogicLayer forward).

Math: y[b,o] = prod_i (1 - (1-atoms[b,i]) * sigmoid(weights[o,i]))
           = exp( sum_i ln( omv[o,i] + v[o,i]*atoms[b,i] ) )
  with v = sigmoid(w), omv = sigmoid(-w) = 1 - v.

Device strategy (8 cores, sharded by OUTPUT FEATURE, 32 o's per core,
atoms replicated):
  * atoms.T lives in SBUF as two (128, 4096) fp32 tiles (i on partitions).
  * For each (o, i-tile): ONE ScalarE activation computes
      logw = Ln(a * v_col + omv_col)   (scale/bias are per-partition APs)
    over the full 4096-wide batch in fp16.
  * TensorE sums logw over the 128 partitions via a one-hot stationary
    (column o = ones) accumulating into PSUM row o, fp32.
  * One final ScalarE Exp over the (32, 4096) PSUM -> y tile -> DMA out.
"""

import os
from contextlib import ExitStack

import numpy as np

B, OUT, IN = 4096, 256, 256
NCORES = 8
O_LOC = OUT // NCORES  # 32 output features per core
PCHUNK = 512  # matmul moving free-dim / PSUM bank size in fp32
LOGW_DT_NAME = os.environ.get("KERNEL_LOGW_DT", "float16")

_COMPILED = {}


def _build_nc():
    import concourse.bacc as bacc
    import concourse.mybir as mybir
    import concourse.tile as tile

    AF = mybir.ActivationFunctionType
    F32 = mybir.dt.float32
    LOGW_DT = getattr(mybir.dt, LOGW_DT_NAME)

    nc = bacc.Bacc(
        "TRN2", target_bir_lowering=False, debug=False, num_devices=NCORES
    )

    aT = nc.dram_tensor("aT", [IN, B], F32, kind="ExternalInput").ap()
    wT = nc.dram_tensor("wT", [IN, O_LOC], F32, kind="ExternalInput").ap()
    sel = nc.dram_tensor("sel", [128, O_LOC * O_LOC], LOGW_DT, kind="ExternalInput").ap()
    y = nc.dram_tensor("y", [O_LOC, B], F32, kind="ExternalOutput").ap()

    NIT = IN // 128  # 2 i-tiles
    NK = B // PCHUNK  # 8 psum bank chunks

    with tile.TileContext(nc) as tc, ExitStack() as es:
        const = es.enter_context(tc.tile_pool(name="const", bufs=1))
        lw_pool = es.enter_context(tc.tile_pool(name="lw", bufs=4))
        ps_pool = es.enter_context(tc.tile_pool(name="ps", bufs=1, space="PSUM"))

        # Small inputs first: weights + selector, so sigmoids can run while
        # the big atoms DMA streams in.
        w_sb = const.tile([128, NIT * O_LOC], F32, name="w_sb", tag="w_sb")
        for it in range(NIT):
            nc.sync.dma_start(
                w_sb[:, it * O_LOC : (it + 1) * O_LOC],
                wT[it * 128 : (it + 1) * 128, :],
            )
        sel_sb = const.tile([128, O_LOC * O_LOC], LOGW_DT, name="sel_sb", tag="sel_sb")
        nc.sync.dma_start(sel_sb[:], sel[:])

        v_sb = const.tile([128, NIT * O_LOC], F32, name="v_sb", tag="v_sb")
        omv_sb = const.tile([128, NIT * O_LOC], F32, name="omv_sb", tag="omv_sb")
        nc.scalar.activation(v_sb[:], w_sb[:], AF.Sigmoid)
        nc.scalar.activation(omv_sb[:], w_sb[:], AF.Sigmoid, scale=-1.0)

        # Atoms: 4 column-chunk DMAs per i-tile so multiple DMA queues run
        # in parallel and the first Ln can start sooner.
        ACH = B // 4
        a_sb = []
        for it in range(NIT):
            t = const.tile([128, B], F32, name=f"a_sb{it}", tag=f"a_sb{it}")
            for q in range(4):
                nc.sync.dma_start(
                    t[:, q * ACH : (q + 1) * ACH],
                    aT[it * 128 : (it + 1) * 128, q * ACH : (q + 1) * ACH],
                )
            a_sb.append(t)

        psum = ps_pool.tile([O_LOC, B], F32, name="psum_S", tag="psum_S")

        for it in range(NIT):
            for o in range(O_LOC):
                c = it * O_LOC + o
                lw = lw_pool.tile([128, B], LOGW_DT, name="lw", tag="lw")
                nc.scalar.activation(
                    lw[:],
                    a_sb[it][:],
                    AF.Ln,
                    bias=omv_sb[:, c : c + 1],
                    scale=v_sb[:, c : c + 1],
                )
                for k in range(NK):
                    nc.tensor.matmul(
                        psum[:, k * PCHUNK : (k + 1) * PCHUNK],
                        lhsT=sel_sb[:, o * O_LOC : (o + 1) * O_LOC],
                        rhs=lw[:, k * PCHUNK : (k + 1) * PCHUNK],
                        start=(it == 0 and o == 0),
                        stop=(it == NIT - 1 and o == O_LOC - 1),
                    )

        # Tail: pipeline Exp chunks with output DMA chunks.
        YCH = B // 2
        y_sb = const.tile([O_LOC, B], F32, name="y_sb", tag="y_sb")
        for q in range(2):
            sl = slice(q * YCH, (q + 1) * YCH)
            nc.scalar.activation(y_sb[:, sl], psum[:, sl], AF.Exp)
            nc.sync.dma_start(y[:, sl], y_sb[:, sl])

    nc.compile()
    return nc


def get_nc():
    if "nc" not in _COMPILED:
        _COMPILED["nc"] = _build_nc()
    return _COMPILED["nc"]


def make_in_maps(atoms: np.ndarray, weights: np.ndarray):
    sel_dt = np.float16 if LOGW_DT_NAME == "float16" else np.float32
    aT = np.ascontiguousarray(atoms.T.astype(np.float32, copy=False))
    sel = np.zeros((128, O_LOC * O_LOC), sel_dt)
    for o in range(O_LOC):
        sel[:, o * O_LOC + o] = 1.0
    in_maps = []
    for c in range(NCORES):
        wT = np.ascontiguousarray(
            weights[c * O_LOC : (c + 1) * O_LOC].T.astype(np.float32, copy=False)
        )
        in_maps.append({"aT": aT, "wT": wT, "sel": sel})
    return in_maps


def run(atoms: np.ndarray, weights: np.ndarray, **spmd_kwargs):
    from concourse.bass_utils import run_bass_kernel_spmd

    nc = get_nc()
    in_maps = make_in_maps(atoms, weights)
    res = run_bass_kernel_spmd(nc, in_maps, core_ids=list(range(NCORES)), **spmd_kwargs)
    yT = np.concatenate([res.results[c]["y"] for c in range(NCORES)], axis=0)
    out = np.ascontiguousarray(yT.T).astype(np.float32, copy=False)
    return out, res


def kernel(atoms: np.ndarray, weights: np.ndarray) -> np.ndarray:
    out, _ = run(atoms, weights)
    return out


# revision 11
# speedup vs baseline: 1.1871x; 1.0041x over previous
"""Trainium2 Bass kernel for nn_LogicLayer (ProductTNorm 'and' LogicLayer forward).

Math: y[b,o] = prod_i (1 - (1-atoms[b,i]) * sigmoid(weights[o,i]))
           = exp( sum_i ln( omv[o,i] + v[o,i]*atoms[b,i] ) )
  with v = sigmoid(w), omv = sigmoid(-w) = 1 - v.

Device strategy (8 cores, sharded by OUTPUT FEATURE, 32 o's per core,
atoms replicated):
  * atoms.T lives in SBUF as two (128, 4096) fp32 tiles (i on partitions).
  * For each (o, i-tile): ONE ScalarE activation computes
      logw = Ln(a * v_col + omv_col)   (scale/bias are per-partition APs)
    over the full 4096-wide batch in fp16.
  * TensorE sums logw over the 128 partitions via a one-hot stationary
    (column o = ones) accumulating into PSUM row o, fp32.
  * One final ScalarE Exp over the (32, 4096) PSUM -> y tile -> DMA out.
"""

import os
from contextlib import ExitStack

import numpy as np

B, OUT, IN = 4096, 256, 256
NCORES = 8
O_LOC = OUT // NCORES  # 32 output features per core
PCHUNK = 512  # matmul moving free-dim / PSUM bank size in fp32
LOGW_DT_NAME = os.environ.get("KERNEL_LOGW_DT", "float16")

_COMPILED = {}


def _build_nc():
    import concourse.bacc as bacc
    import concourse.mybir as mybir
    import concourse.tile as tile

    AF = mybir.ActivationFunctionType
    F32 = mybir.dt.float32
    LOGW_DT = getattr(mybir.dt, LOGW_DT_NAME)

    nc = bacc.Bacc(
        "TRN2", target_bir_lowering=False, debug=False, num_devices=NCORES
    )

    aT = nc.dram_tensor("aT", [IN, B], F32, kind="ExternalInput").ap()
    wT = nc.dram_tensor("wT", [IN, O_LOC], F32, kind="ExternalInput").ap()
    sel = nc.dram_tensor("sel", [128, O_LOC * O_LOC], LOGW_DT, kind="ExternalInput").ap()
    y = nc.dram_tensor("y", [O_LOC, B], F32, kind="ExternalOutput").ap()

    NIT = IN // 128  # 2 i-tiles
    NK = B // PCHUNK  # 8 psum bank chunks

    with tile.TileContext(nc) as tc, ExitStack() as es:
        const = es.enter_context(tc.tile_pool(name="const", bufs=1))
        lw_pool = es.enter_context(tc.tile_pool(name="lw", bufs=4))
        ps_pool = es.enter_context(tc.tile_pool(name="ps", bufs=1, space="PSUM"))

        # Small inputs first: weights + selector, so sigmoids can run while
        # the big atoms DMA streams in.
        w_sb = const.tile([128, NIT * O_LOC], F32, name="w_sb", tag="w_sb")
        for it in range(NIT):
            nc.sync.dma_start(
                w_sb[:, it * O_LOC : (it + 1) * O_LOC],
                wT[it * 128 : (it + 1) * 128, :],
            )
        sel_sb = const.tile([128, O_LOC * O_LOC], LOGW_DT, name="sel_sb", tag="sel_sb")
        nc.sync.dma_start(sel_sb[:], sel[:])

        v_sb = const.tile([128, NIT * O_LOC], F32, name="v_sb", tag="v_sb")
        omv_sb = const.tile([128, NIT * O_LOC], F32, name="omv_sb", tag="omv_sb")
        nc.scalar.activation(v_sb[:], w_sb[:], AF.Sigmoid)
        nc.scalar.activation(omv_sb[:], w_sb[:], AF.Sigmoid, scale=-1.0)

        # Atoms: 8 column-chunk DMAs per i-tile. Eight chunks span all DMA
        # rings, so tile0 (needed by the first Ln) streams at full HBM
        # bandwidth before tile1's chunks queue up behind it.
        ACH = B // 8
        a_sb = []
        for it in range(NIT):
            t = const.tile([128, B], F32, name=f"a_sb{it}", tag=f"a_sb{it}")
            for q in range(8):
                nc.sync.dma_start(
                    t[:, q * ACH : (q + 1) * ACH],
                    aT[it * 128 : (it + 1) * 128, q * ACH : (q + 1) * ACH],
                )
            a_sb.append(t)

        psum = ps_pool.tile([O_LOC, B], F32, name="psum_S", tag="psum_S")

        for it in range(NIT):
            for o in range(O_LOC):
                c = it * O_LOC + o
                lw = lw_pool.tile([128, B], LOGW_DT, name="lw", tag="lw")
                nc.scalar.activation(
                    lw[:],
                    a_sb[it][:],
                    AF.Ln,
                    bias=omv_sb[:, c : c + 1],
                    scale=v_sb[:, c : c + 1],
                )
                for k in range(NK):
                    nc.tensor.matmul(
                        psum[:, k * PCHUNK : (k + 1) * PCHUNK],
                        lhsT=sel_sb[:, o * O_LOC : (o + 1) * O_LOC],
                        rhs=lw[:, k * PCHUNK : (k + 1) * PCHUNK],
                        start=(it == 0 and o == 0),
                        stop=(it == NIT - 1 and o == O_LOC - 1),
                    )

        # Tail: pipeline Exp chunks with output DMA chunks.
        YCH = B // 2
        y_sb = const.tile([O_LOC, B], F32, name="y_sb", tag="y_sb")
        for q in range(2):
            sl = slice(q * YCH, (q + 1) * YCH)
            nc.scalar.activation(y_sb[:, sl], psum[:, sl], AF.Exp)
            nc.sync.dma_start(y[:, sl], y_sb[:, sl])

    nc.compile()
    return nc


def get_nc():
    if "nc" not in _COMPILED:
        _COMPILED["nc"] = _build_nc()
    return _COMPILED["nc"]


def make_in_maps(atoms: np.ndarray, weights: np.ndarray):
    sel_dt = np.float16 if LOGW_DT_NAME == "float16" else np.float32
    aT = np.ascontiguousarray(atoms.T.astype(np.float32, copy=False))
    sel = np.zeros((128, O_LOC * O_LOC), sel_dt)
    for o in range(O_LOC):
        sel[:, o * O_LOC + o] = 1.0
    in_maps = []
    for c in range(NCORES):
        wT = np.ascontiguousarray(
            weights[c * O_LOC : (c + 1) * O_LOC].T.astype(np.float32, copy=False)
        )
        in_maps.append({"aT": aT, "wT": wT, "sel": sel})
    return in_maps


def run(atoms: np.ndarray, weights: np.ndarray, **spmd_kwargs):
    from concourse.bass_utils import run_bass_kernel_spmd

    nc = get_nc()
    in_maps = make_in_maps(atoms, weights)
    res = run_bass_kernel_spmd(nc, in_maps, core_ids=list(range(NCORES)), **spmd_kwargs)
    yT = np.concatenate([res.results[c]["y"] for c in range(NCORES)], axis=0)
    out = np.ascontiguousarray(yT.T).astype(np.float32, copy=False)
    return out, res


def kernel(atoms: np.ndarray, weights: np.ndarray) -> np.ndarray:
    out, _ = run(atoms, weights)
    return out
